# revision 20
# baseline (speedup 1.0000x reference)
"""Causal single-head attention on 8 Trainium2 NeuronCores.

Problem: x[4, 4096, 1024], Wq/Wk/Wv[1024, 64] ->
  out = softmax(causal(Q K^T / 8)) V   per batch, fp32.

Sharding: core i handles batch b = i//2 with query-chunk parity p = i%2 at
256-query granularity: core p owns global 256-chunks {2j+p : j=0..7}. Both
cores of a pair load the full x[b] (transposed on host to [C, T]) and
compute full K/V; causal work is balanced by interleaving query chunks.

All matmul operands are bf16 (fp32 accumulate in PSUM). Scores are computed
transposed (S^T[k, q]) with contraction HS=64, so two key tiles run
concurrently in the PE array via partition row-groups (base partition 0 and
64). Each flush step covers a "quad" (4 key tiles = 512 keys x 256 queries)
in one [128, 4, 256] PSUM tile: one exp ACTIVATE per quad. Causality on the
diagonal quad is applied *additively in PSUM before the exp*: 4 extra
matmuls with an identity stationary accumulate -30000 onto masked score
positions, so exp underflows to exactly 0 and no post-exp mask op (or its
latency) exists. V carries a ones column so softmax row-sums accumulate
with P@V; the unnormalized out^T [65, q] is DMA'd straight out of PSUM and
division + transpose happen on host.

Schedule: a short warm-up matmul burst keeps the PE HAM clock-gate at
2.4 GHz through the DMA-limited start; x chunks stream on one strictly
ordered queue so chunk s always lands before chunk s+1 bytes; chunk s's
projections are emitted *inside* the slot s-1 flush (right before the
diagonal PV) so the last exp latency is hidden by projection matmuls.
"""

import numpy as np
import ml_dtypes

import concourse.bacc as bacc
import concourse.mybir as mybir
import concourse.tile as tile
from concourse.bass_utils import run_bass_kernel_spmd

# Problem dims
B, T, C, HS = 4, 4096, 1024, 64
P = 128           # partitions
CH = 512          # projection chunk width
CHA = 256         # attention query-slot width
NSLOT = 8         # query slots per core (256 wide)
CSUB = C // P     # 8 contraction subtiles
NKT = T // P      # 32 key tiles total
NPAIR = NKT // 2  # 16 key-tile pairs
NWARM = 30        # PE warm-up matmuls (HAM clock-gate release)

BF16 = mybir.dt.bfloat16
# key tile (within quad) held by each st/pt slot: slot s <-> tile 4w+QORD[s]
QORD = (1, 3, 0, 2)


def _build_program():
    nc = bacc.Bacc("TRN2")
    f32 = mybir.dt.float32
    EXP = mybir.ActivationFunctionType.Exp

    xT = nc.dram_tensor("xT", [C, T], BF16, kind="ExternalInput").ap()
    wqk = nc.dram_tensor("wqk", [C, 2 * HS], BF16, kind="ExternalInput").ap()
    wv = nc.dram_tensor("wv", [C, HS], BF16, kind="ExternalInput").ap()
    maskadd_d = nc.dram_tensor("maskadd", [P, 4, CHA], BF16, kind="ExternalInput").ap()
    pmask_d = nc.dram_tensor("pmask", [HS, CHA], mybir.dt.uint8, kind="ExternalInput").ap()
    eye_d = nc.dram_tensor("eye", [P, P], BF16, kind="ExternalInput").ap()
    out_d = nc.dram_tensor("out", [HS + 1, NSLOT * CHA], f32, kind="ExternalOutput").ap()

    xT_r = xT.rearrange("(co ci) t -> ci co t", ci=P)      # [128, 8, 4096]
    wqk_r = wqk.rearrange("(co ci) m -> ci co m", ci=P)    # [128, 8, 128]
    wv_r = wv.rearrange("(co ci) m -> ci co m", ci=P)      # [128, 8, 64]

    with tile.TileContext(nc) as tc:
        with (
            tc.tile_pool(name="const", bufs=1) as const_pool,
            tc.tile_pool(name="persist", bufs=1) as persist,
            tc.tile_pool(name="xina", bufs=2) as xpool_a,
            tc.tile_pool(name="xinb", bufs=3) as xpool_b,
            tc.tile_pool(name="pt", bufs=6) as pt_pool,
            tc.tile_pool(name="osb", bufs=2) as osb_pool,
            tc.tile_pool(name="qk_ps", bufs=2, space="PSUM") as qk_pool,
            tc.tile_pool(name="v_ps", bufs=1, space="PSUM") as v_pool,
            tc.tile_pool(name="st_ps", bufs=2, space="PSUM") as st_ps,
            tc.tile_pool(name="ot_ps", bufs=1, space="PSUM") as ot_ps,
        ):
            # ---- constants ----------------------------------------------
            # wqk gates the very first projection: give it (and the other
            # small consts) the otherwise-idle Activation queue, leaving the
            # Sync queue exclusively to the strictly-ordered x stream.
            wqk_sb = const_pool.tile([P, CSUB, 2 * HS], BF16)
            wv_sb = const_pool.tile([P, CSUB, HS], BF16)
            maskadd_sb = const_pool.tile([P, 4, CHA], BF16)
            pmask_sb = const_pool.tile([HS, CHA], mybir.dt.uint8)
            eye_sb = const_pool.tile([P, P], BF16)
            nc.scalar.dma_start(wqk_sb[:], wqk_r)
            nc.scalar.dma_start(wv_sb[:], wv_r)
            nc.scalar.dma_start(pmask_sb[:], pmask_d)
            # needed only from the first flush (~iter 1): keep them off the
            # critical const queue
            nc.gpsimd.dma_start(eye_sb[:], eye_d)
            nc.gpsimd.dma_start(maskadd_sb[:], maskadd_d)

            # warm-up operand: all-ones bf16, no DMA dependency
            wtile = const_pool.tile([P, CHA], BF16)
            nc.vector.memset(wtile[:].bitcast(mybir.dt.uint16), 0x3F80)

            # ---- persistent SBUF ---------------------------------------
            # K^T pairs: [0:64, u, :] = tile 2u, [64:128, u, :] = tile 2u+1
            kt_all = persist.tile([P, NPAIR, P], BF16)
            # Q^T, pair-major: slot s at [:, s//2, (s%2)*256:(s%2+1)*256]
            qt_slot = persist.tile([P, NSLOT // 2, 2 * CHA], BF16)
            v_all = persist.tile([P, NKT, HS + 1], BF16)     # V with ones col
            nc.vector.memset(
                v_all[:, :, HS : HS + 1].bitcast(mybir.dt.uint16), 0x3F80
            )

            # ---- x stream: one queue, strict chunk order ----------------
            # chunks 0/1 in fine-grained 1KB-line DMAs (early start for the
            # first projections); chunks 2..7 as 2 MB chunk-pair DMAs whose
            # 2KB-per-partition lines issue ~4x cheaper per byte.
            xtile = {}
            c0 = xpool_a.tile([P, CSUB, CH], BF16, tag="xa")
            for q in range(4):
                nc.sync.dma_start(c0[:, 2 * q : 2 * q + 2, :],
                                  xT_r[:, 2 * q : 2 * q + 2, 0:CH])
            c1 = xpool_a.tile([P, CSUB, CH], BF16, tag="xa")
            for q in range(2):
                nc.sync.dma_start(c1[:, 4 * q : 4 * q + 4, :],
                                  xT_r[:, 4 * q : 4 * q + 4, CH : 2 * CH])
            xtile[0], xtile[1] = (c0, 0), (c1, 0)
            for cp in (1, 2, 3):
                xc = xpool_b.tile([P, CSUB, 2 * CH], BF16, tag="xb")
                nc.sync.dma_start(xc[:], xT_r[:, :, 2 * cp * CH : 2 * (cp + 1) * CH])
                xtile[2 * cp] = (xc, 0)
                xtile[2 * cp + 1] = (xc, CH)

            # ---- PE warm-up --------------------------------------------
            # ~3.2us of dummy matmuls while x streams in: holds the PE HAM
            # activity window busy so real matmuls run at 2.4 GHz from the
            # first chunk (otherwise the clock gate stays at 1.2 GHz for the
            # whole DMA-limited start).
            wm = st_ps.tile([P, 4, CHA], f32, tag="st")
            for k in range(NWARM):
                nc.tensor.matmul(
                    wm[:, k % 4, :], lhsT=wtile[:, 0:P], rhs=wtile[:],
                    start=True, stop=True,
                )

            # ---- per-chunk emission helpers -----------------------------
            def emit_qkproj(s):
                xc, xlo = xtile[s]
                qk_ps = qk_pool.tile([P, CH], f32, tag="qk")
                for cs in range(CSUB):
                    nc.tensor.matmul(
                        qk_ps[:],
                        lhsT=wqk_sb[:, cs, :],
                        rhs=xc[:, cs, xlo : xlo + CH],
                        start=(cs == 0),
                        stop=(cs == CSUB - 1),
                    )
                return qk_ps

            def emit_vproj(s):
                xc, xlo = xtile[s]
                v_ps = v_pool.tile([P, 4, HS], f32, tag="v")
                for tt in range(4):
                    for cs in range(CSUB):
                        nc.tensor.matmul(
                            v_ps[:, tt, :],
                            lhsT=xc[:, cs, xlo + tt * P : xlo + (tt + 1) * P],
                            rhs=wv_sb[:, cs, :],
                            start=(cs == 0),
                            stop=(cs == CSUB - 1),
                        )
                return v_ps

            def emit_copies(s, qk_ps, v_ps):
                # Q select for this core's parity (runtime-chosen via pmask)
                qlo = (s % 2) * CHA
                for hb in (0, HS):
                    nc.vector.tensor_copy(
                        qt_slot[hb : hb + HS, s // 2, qlo : qlo + CHA],
                        qk_ps[0:HS, 0:CHA],
                    )
                    nc.vector.copy_predicated(
                        qt_slot[hb : hb + HS, s // 2, qlo : qlo + CHA],
                        pmask_sb[:],
                        qk_ps[0:HS, CHA:CH],
                    )
                # K^T pair split: chunk s holds key tiles 4s..4s+3
                ksrc = qk_ps[HS:P, :].rearrange(
                    "p (i par c) -> p i par c", i=2, par=2, c=P
                )
                nc.vector.tensor_copy(
                    kt_all[0:HS, 2 * s : 2 * s + 2, :], ksrc[:, :, 0, :]
                )
                nc.vector.tensor_copy(
                    kt_all[HS:P, 2 * s : 2 * s + 2, :], ksrc[:, :, 1, :]
                )
                nc.vector.tensor_copy(
                    v_all[:, 4 * s : 4 * s + 4, 0:HS], v_ps[:]
                )

            def emit_squad(j, w):
                # scores S^T for quad w of slot j; diagonal quad (w == j)
                # gets the additive causal mask accumulated in PSUM so exp
                # underflows to 0 on masked positions.
                qlo = (j % 2) * CHA
                st = st_ps.tile([P, 4, CHA], f32, tag="st")
                diag = w == j
                for issue, (slot, o) in enumerate(
                    ((0, 1), (2, 0), (1, 3), (3, 2))
                ):
                    u, hi = divmod(o, 2)  # pair 2w+u, row half hi
                    hb = HS if hi else 0
                    nc.tensor.matmul(
                        st[:, slot, :],
                        lhsT=kt_all[hb : hb + HS, 2 * w + u, :],
                        rhs=qt_slot[hb : hb + HS, j // 2, qlo : qlo + CHA],
                        start=(issue < 2),
                        stop=(issue >= 2) and not diag,
                        skip_group_check=True,
                    )
                if diag:
                    # additive causal mask, 2 slots per matmul (PSUM-bank
                    # sized N=512; identity stationary loaded once)
                    for h in range(2):
                        nc.tensor.matmul(
                            st[:, 2 * h : 2 * h + 2, :],
                            lhsT=eye_sb[:],
                            rhs=maskadd_sb[:, 2 * h : 2 * h + 2, :],
                            start=False,
                            stop=True,
                            skip_group_check=True,
                        )
                pt = pt_pool.tile([P, 4, CHA], BF16, tag="pt")
                nc.scalar.activation(pt[:], st[:], EXP, scale=float(HS) ** -0.5)
                return pt

            def emit_pv(j, w, pt, ot):
                for slot, o in ((0, 1), (2, 0), (1, 3), (3, 2)):
                    nc.tensor.matmul(
                        ot[0 : HS + 1, :],
                        lhsT=v_all[:, 4 * w + o, :],
                        rhs=pt[:, slot, :],
                        start=(w == 0 and slot == 0),
                        stop=(w == j and slot == 3),
                    )

            # ---- main loop ---------------------------------------------
            # iter s: flush slot j = s-1 while chunk s's projections are
            # emitted right before the last two PVs (hides the final exps'
            # latency behind projection matmuls). The first AHEAD[j] score
            # quads of slot j run *ahead* in iter j itself, right after
            # chunk j's copies, so the scalar engine's exp stream stays fed
            # across iteration boundaries and the final (projection-less)
            # iteration shrinks.
            AHEAD = [1, 2, 2, 2, 2, 2, 2, 2]
            pts = {}
            for s in range(NSLOT + 1):
                j = s - 1
                if j >= 0:
                    ot = ot_ps.tile([P, CHA], f32, tag="ot")
                    for w in range(AHEAD[j], j + 1):
                        pts[(j, w)] = emit_squad(j, w)
                        if w - 2 >= 0:
                            emit_pv(j, w - 2, pts[(j, w - 2)], ot)
                if s < NSLOT:
                    qk_ps = emit_qkproj(s)
                    v_ps = emit_vproj(s)
                    # copies before the trailing PVs: the vector queue must
                    # reach chunk s's qt select before slot s's first S
                    emit_copies(s, qk_ps, v_ps)
                if j >= 0:
                    if j - 1 >= 0:
                        emit_pv(j, j - 1, pts[(j, j - 1)], ot)
                    emit_pv(j, j, pts[(j, j)], ot)
                    # unnormalized out^T + sums row; host finishes
                    o_sb = osb_pool.tile([HS + 1, CHA], f32, tag="osb")
                    nc.vector.tensor_copy(o_sb[:], ot[0 : HS + 1, :])
                    nc.sync.dma_start(
                        out_d[:, j * CHA : (j + 1) * CHA], o_sb[:]
                    )
                if s < NSLOT:
                    for w in range(AHEAD[s]):
                        pts[(s, w)] = emit_squad(s, w)

    nc.compile()
    return nc


_CACHE = {}


def _get_program():
    if "nc" not in _CACHE:
        _CACHE["nc"] = _build_program()
    return _CACHE["nc"]


def _host_inputs(x, Wk, Wq, Wv):
    bf = ml_dtypes.bfloat16
    x = np.asarray(x, dtype=np.float32)
    wqk = np.ascontiguousarray(
        np.concatenate([np.asarray(Wq), np.asarray(Wk)], axis=1), dtype=np.float32
    ).astype(bf)
    wv = np.ascontiguousarray(np.asarray(Wv), dtype=np.float32).astype(bf)
    eye = np.eye(P, dtype=np.float32).astype(bf)

    xTs = [np.ascontiguousarray(x[b].T).astype(bf) for b in range(B)]

    # kept[i, q, c] = 1 iff c >= 128*QORD[q] + i - 256 p   (diagonal quad);
    # additive mask: -30000 where not kept, 0 where kept (pre-exp, in PSUM)
    ii = np.arange(P)[:, None, None]
    qq = np.array(QORD)[None, :, None]
    cc = np.arange(CHA)[None, None, :]
    maskadds = [
        np.where(cc >= (128 * qq + ii - 256 * p), 0.0, -30000.0).astype(bf)
        for p in range(2)
    ]
    pmasks = [np.full((HS, CHA), p, dtype=np.uint8) for p in range(2)]

    in_maps = []
    for core in range(2 * B):
        b, p = core // 2, core % 2
        in_maps.append(
            {
                "xT": xTs[b],
                "wqk": wqk,
                "wv": wv,
                "maskadd": maskadds[p],
                "pmask": pmasks[p],
                "eye": eye,
            }
        )
    return in_maps


def _assemble(results):
    out = np.empty((B, T, HS), dtype=np.float32)
    for core in range(2 * B):
        b, p = core // 2, core % 2
        oc = np.asarray(results[core]["out"], dtype=np.float32)  # [65, 2048]
        for j in range(NSLOT):
            g = 2 * j + p
            blk = oc[:, j * CHA : (j + 1) * CHA]
            out[b, g * CHA : (g + 1) * CHA, :] = (blk[0:HS] / blk[HS : HS + 1]).T
    return out


def run(x, Wk, Wq, Wv, trace=False):
    nc = _get_program()
    in_maps = _host_inputs(x, Wk, Wq, Wv)
    res = run_bass_kernel_spmd(nc, in_maps, list(range(2 * B)), trace=trace)
    return _assemble(res.results), res


def kernel(x, Wk, Wq, Wv):
    out, _ = run(x, Wk, Wq, Wv)
    return out


# revision 22
# speedup vs baseline: 1.0074x; 1.0074x over previous
"""Causal single-head attention on 8 Trainium2 NeuronCores.

Problem: x[4, 4096, 1024], Wq/Wk/Wv[1024, 64] ->
  out = softmax(causal(Q K^T / 8)) V   per batch, fp32.

Sharding: core i handles batch b = i//2 with query-chunk parity p = i%2 at
256-query granularity: core p owns global 256-chunks {2j+p : j=0..7}. Both
cores of a pair load the full x[b] (transposed on host to [C, T]) and
compute full K/V; causal work is balanced by interleaving query chunks.

All matmul operands are bf16 (fp32 accumulate in PSUM). Scores are computed
transposed (S^T[k, q]) with contraction HS=64, so two key tiles run
concurrently in the PE array via partition row-groups (base partition 0 and
64). Each flush step covers a "quad" (4 key tiles = 512 keys x 256 queries)
in one [128, 4, 256] PSUM tile: one exp ACTIVATE per quad. Causality on the
diagonal quad is applied *additively in PSUM before the exp*: 4 extra
matmuls with an identity stationary accumulate -30000 onto masked score
positions, so exp underflows to exactly 0 and no post-exp mask op (or its
latency) exists. V carries a ones column so softmax row-sums accumulate
with P@V; the unnormalized out^T [65, q] is DMA'd straight out of PSUM and
division + transpose happen on host.

Schedule: a short warm-up matmul burst keeps the PE HAM clock-gate at
2.4 GHz through the DMA-limited start; x chunks stream on one strictly
ordered queue so chunk s always lands before chunk s+1 bytes; chunk s's
projections are emitted *inside* the slot s-1 flush (right before the
diagonal PV) so the last exp latency is hidden by projection matmuls.
"""

import numpy as np
import ml_dtypes

import concourse.bacc as bacc
import concourse.mybir as mybir
import concourse.tile as tile
from concourse.bass_utils import run_bass_kernel_spmd

# Problem dims
B, T, C, HS = 4, 4096, 1024, 64
P = 128           # partitions
CH = 512          # projection chunk width
CHA = 256         # attention query-slot width
NSLOT = 8         # query slots per core (256 wide)
CSUB = C // P     # 8 contraction subtiles
NKT = T // P      # 32 key tiles total
NPAIR = NKT // 2  # 16 key-tile pairs
NWARM = 22        # PE warm-up matmuls (HAM clock-gate release)

BF16 = mybir.dt.bfloat16
# key tile (within quad) held by each st/pt slot: slot s <-> tile 4w+QORD[s]
QORD = (1, 3, 0, 2)


def _build_program():
    nc = bacc.Bacc("TRN2")
    f32 = mybir.dt.float32
    EXP = mybir.ActivationFunctionType.Exp

    xT = nc.dram_tensor("xT", [C, T], BF16, kind="ExternalInput").ap()
    wqk = nc.dram_tensor("wqk", [C, 2 * HS], BF16, kind="ExternalInput").ap()
    wv = nc.dram_tensor("wv", [C, HS], BF16, kind="ExternalInput").ap()
    maskadd_d = nc.dram_tensor("maskadd", [P, 4, CHA], BF16, kind="ExternalInput").ap()
    pmask_d = nc.dram_tensor("pmask", [HS, CHA], mybir.dt.uint8, kind="ExternalInput").ap()
    eye_d = nc.dram_tensor("eye", [P, P], BF16, kind="ExternalInput").ap()
    out_d = nc.dram_tensor("out", [HS + 1, NSLOT * CHA], f32, kind="ExternalOutput").ap()

    xT_r = xT.rearrange("(co ci) t -> ci co t", ci=P)      # [128, 8, 4096]
    wqk_r = wqk.rearrange("(co ci) m -> ci co m", ci=P)    # [128, 8, 128]
    wv_r = wv.rearrange("(co ci) m -> ci co m", ci=P)      # [128, 8, 64]

    with tile.TileContext(nc) as tc:
        with (
            tc.tile_pool(name="const", bufs=1) as const_pool,
            tc.tile_pool(name="persist", bufs=1) as persist,
            tc.tile_pool(name="xina", bufs=2) as xpool_a,
            tc.tile_pool(name="xinb", bufs=3) as xpool_b,
            tc.tile_pool(name="pt", bufs=6) as pt_pool,
            tc.tile_pool(name="osb", bufs=2) as osb_pool,
            tc.tile_pool(name="qk_ps", bufs=2, space="PSUM") as qk_pool,
            tc.tile_pool(name="v_ps", bufs=1, space="PSUM") as v_pool,
            tc.tile_pool(name="st_ps", bufs=2, space="PSUM") as st_ps,
            tc.tile_pool(name="ot_ps", bufs=1, space="PSUM") as ot_ps,
        ):
            # ---- constants ----------------------------------------------
            # wqk gates the very first projection: give it (and the other
            # small consts) the otherwise-idle Activation queue, leaving the
            # Sync queue exclusively to the strictly-ordered x stream.
            wqk_sb = const_pool.tile([P, CSUB, 2 * HS], BF16)
            wv_sb = const_pool.tile([P, CSUB, HS], BF16)
            maskadd_sb = const_pool.tile([P, 4, CHA], BF16)
            pmask_sb = const_pool.tile([HS, CHA], mybir.dt.uint8)
            eye_sb = const_pool.tile([P, P], BF16)
            nc.scalar.dma_start(wqk_sb[:], wqk_r)
            nc.scalar.dma_start(wv_sb[:], wv_r)
            nc.scalar.dma_start(pmask_sb[:], pmask_d)
            # needed only from the first flush (~iter 1): keep them off the
            # critical const queue
            nc.gpsimd.dma_start(eye_sb[:], eye_d)
            nc.gpsimd.dma_start(maskadd_sb[:], maskadd_d)

            # warm-up operand: all-ones bf16, no DMA dependency
            wtile = const_pool.tile([P, CHA], BF16)
            nc.vector.memset(wtile[:].bitcast(mybir.dt.uint16), 0x3F80)

            # ---- persistent SBUF ---------------------------------------
            # K^T pairs: [0:64, u, :] = tile 2u, [64:128, u, :] = tile 2u+1
            kt_all = persist.tile([P, NPAIR, P], BF16)
            # Q^T, pair-major: slot s at [:, s//2, (s%2)*256:(s%2+1)*256]
            qt_slot = persist.tile([P, NSLOT // 2, 2 * CHA], BF16)
            v_all = persist.tile([P, NKT, HS + 1], BF16)     # V with ones col
            nc.vector.memset(
                v_all[:, :, HS : HS + 1].bitcast(mybir.dt.uint16), 0x3F80
            )

            # ---- x stream: one queue, strict chunk order ----------------
            # chunks 0/1 in fine-grained 1KB-line DMAs (early start for the
            # first projections); chunks 2..7 as 2 MB chunk-pair DMAs whose
            # 2KB-per-partition lines issue ~4x cheaper per byte.
            xtile = {}
            c0 = xpool_a.tile([P, CSUB, CH], BF16, tag="xa")
            for q in range(4):
                nc.sync.dma_start(c0[:, 2 * q : 2 * q + 2, :],
                                  xT_r[:, 2 * q : 2 * q + 2, 0:CH])
            c1 = xpool_a.tile([P, CSUB, CH], BF16, tag="xa")
            for q in range(2):
                nc.sync.dma_start(c1[:, 4 * q : 4 * q + 4, :],
                                  xT_r[:, 4 * q : 4 * q + 4, CH : 2 * CH])
            xtile[0], xtile[1] = (c0, 0), (c1, 0)
            for cp in (1, 2, 3):
                xc = xpool_b.tile([P, CSUB, 2 * CH], BF16, tag="xb")
                nc.sync.dma_start(xc[:], xT_r[:, :, 2 * cp * CH : 2 * (cp + 1) * CH])
                xtile[2 * cp] = (xc, 0)
                xtile[2 * cp + 1] = (xc, CH)

            # ---- PE warm-up --------------------------------------------
            # ~3.2us of dummy matmuls while x streams in: holds the PE HAM
            # activity window busy so real matmuls run at 2.4 GHz from the
            # first chunk (otherwise the clock gate stays at 1.2 GHz for the
            # whole DMA-limited start).
            wm = st_ps.tile([P, 4, CHA], f32, tag="st")
            for k in range(NWARM):
                nc.tensor.matmul(
                    wm[:, k % 4, :], lhsT=wtile[:, 0:P], rhs=wtile[:],
                    start=True, stop=True,
                )

            # ---- per-chunk emission helpers -----------------------------
            def emit_qkproj(s):
                xc, xlo = xtile[s]
                qk_ps = qk_pool.tile([P, CH], f32, tag="qk")
                for cs in range(CSUB):
                    nc.tensor.matmul(
                        qk_ps[:],
                        lhsT=wqk_sb[:, cs, :],
                        rhs=xc[:, cs, xlo : xlo + CH],
                        start=(cs == 0),
                        stop=(cs == CSUB - 1),
                    )
                    if s == 0:
                        # chunk 0 is DMA-starved: keep the PE HAM activity
                        # window busy between per-cs slices so the clock
                        # gate never re-throttles during the start
                        for _ in range(2):
                            nc.tensor.matmul(
                                wm[:, cs % 4, :], lhsT=wtile[:, 0:P],
                                rhs=wtile[:], start=True, stop=True,
                            )
                return qk_ps

            def emit_vproj(s):
                xc, xlo = xtile[s]
                v_ps = v_pool.tile([P, 4, HS], f32, tag="v")
                for tt in range(4):
                    for cs in range(CSUB):
                        nc.tensor.matmul(
                            v_ps[:, tt, :],
                            lhsT=xc[:, cs, xlo + tt * P : xlo + (tt + 1) * P],
                            rhs=wv_sb[:, cs, :],
                            start=(cs == 0),
                            stop=(cs == CSUB - 1),
                        )
                return v_ps

            def emit_copies(s, qk_ps, v_ps):
                # Q select for this core's parity (runtime-chosen via pmask)
                qlo = (s % 2) * CHA
                for hb in (0, HS):
                    nc.vector.tensor_copy(
                        qt_slot[hb : hb + HS, s // 2, qlo : qlo + CHA],
                        qk_ps[0:HS, 0:CHA],
                    )
                    nc.vector.copy_predicated(
                        qt_slot[hb : hb + HS, s // 2, qlo : qlo + CHA],
                        pmask_sb[:],
                        qk_ps[0:HS, CHA:CH],
                    )
                # K^T pair split: chunk s holds key tiles 4s..4s+3
                ksrc = qk_ps[HS:P, :].rearrange(
                    "p (i par c) -> p i par c", i=2, par=2, c=P
                )
                nc.vector.tensor_copy(
                    kt_all[0:HS, 2 * s : 2 * s + 2, :], ksrc[:, :, 0, :]
                )
                nc.vector.tensor_copy(
                    kt_all[HS:P, 2 * s : 2 * s + 2, :], ksrc[:, :, 1, :]
                )
                nc.vector.tensor_copy(
                    v_all[:, 4 * s : 4 * s + 4, 0:HS], v_ps[:]
                )

            def emit_squad(j, w):
                # scores S^T for quad w of slot j; diagonal quad (w == j)
                # gets the additive causal mask accumulated in PSUM so exp
                # underflows to 0 on masked positions.
                qlo = (j % 2) * CHA
                st = st_ps.tile([P, 4, CHA], f32, tag="st")
                diag = w == j
                for issue, (slot, o) in enumerate(
                    ((0, 1), (2, 0), (1, 3), (3, 2))
                ):
                    u, hi = divmod(o, 2)  # pair 2w+u, row half hi
                    hb = HS if hi else 0
                    nc.tensor.matmul(
                        st[:, slot, :],
                        lhsT=kt_all[hb : hb + HS, 2 * w + u, :],
                        rhs=qt_slot[hb : hb + HS, j // 2, qlo : qlo + CHA],
                        start=(issue < 2),
                        stop=(issue >= 2) and not diag,
                        skip_group_check=True,
                    )
                if diag:
                    # additive causal mask, 2 slots per matmul (PSUM-bank
                    # sized N=512; identity stationary loaded once)
                    for h in range(2):
                        nc.tensor.matmul(
                            st[:, 2 * h : 2 * h + 2, :],
                            lhsT=eye_sb[:],
                            rhs=maskadd_sb[:, 2 * h : 2 * h + 2, :],
                            start=False,
                            stop=True,
                            skip_group_check=True,
                        )
                pt = pt_pool.tile([P, 4, CHA], BF16, tag="pt")
                nc.scalar.activation(pt[:], st[:], EXP, scale=float(HS) ** -0.5)
                return pt

            def emit_pv(j, w, pt, ot):
                for slot, o in ((0, 1), (2, 0), (1, 3), (3, 2)):
                    nc.tensor.matmul(
                        ot[0 : HS + 1, :],
                        lhsT=v_all[:, 4 * w + o, :],
                        rhs=pt[:, slot, :],
                        start=(w == 0 and slot == 0),
                        stop=(w == j and slot == 3),
                    )

            # ---- main loop ---------------------------------------------
            # iter s: flush slot j = s-1 while chunk s's projections are
            # emitted right before the last two PVs (hides the final exps'
            # latency behind projection matmuls). The first AHEAD[j] score
            # quads of slot j run *ahead* in iter j itself, right after
            # chunk j's copies, so the scalar engine's exp stream stays fed
            # across iteration boundaries and the final (projection-less)
            # iteration shrinks.
            AHEAD = [1, 2, 2, 2, 2, 2, 2, 2]
            pts = {}
            for s in range(NSLOT + 1):
                j = s - 1
                if j >= 0:
                    ot = ot_ps.tile([P, CHA], f32, tag="ot")
                    for w in range(AHEAD[j], j + 1):
                        pts[(j, w)] = emit_squad(j, w)
                        if w - 2 >= 0:
                            emit_pv(j, w - 2, pts[(j, w - 2)], ot)
                if s < NSLOT:
                    qk_ps = emit_qkproj(s)
                    v_ps = emit_vproj(s)
                    # copies before the trailing PVs: the vector queue must
                    # reach chunk s's qt select before slot s's first S
                    emit_copies(s, qk_ps, v_ps)
                if j >= 0:
                    if j - 1 >= 0:
                        emit_pv(j, j - 1, pts[(j, j - 1)], ot)
                    emit_pv(j, j, pts[(j, j)], ot)
                    # unnormalized out^T + sums row; host finishes
                    o_sb = osb_pool.tile([HS + 1, CHA], f32, tag="osb")
                    nc.vector.tensor_copy(o_sb[:], ot[0 : HS + 1, :])
                    nc.sync.dma_start(
                        out_d[:, j * CHA : (j + 1) * CHA], o_sb[:]
                    )
                if s < NSLOT:
                    for w in range(AHEAD[s]):
                        pts[(s, w)] = emit_squad(s, w)

    nc.compile()
    return nc


_CACHE = {}


def _get_program():
    if "nc" not in _CACHE:
        _CACHE["nc"] = _build_program()
    return _CACHE["nc"]


def _host_inputs(x, Wk, Wq, Wv):
    bf = ml_dtypes.bfloat16
    x = np.asarray(x, dtype=np.float32)
    wqk = np.ascontiguousarray(
        np.concatenate([np.asarray(Wq), np.asarray(Wk)], axis=1), dtype=np.float32
    ).astype(bf)
    wv = np.ascontiguousarray(np.asarray(Wv), dtype=np.float32).astype(bf)
    eye = np.eye(P, dtype=np.float32).astype(bf)

    xTs = [np.ascontiguousarray(x[b].T).astype(bf) for b in range(B)]

    # kept[i, q, c] = 1 iff c >= 128*QORD[q] + i - 256 p   (diagonal quad);
    # additive mask: -30000 where not kept, 0 where kept (pre-exp, in PSUM)
    ii = np.arange(P)[:, None, None]
    qq = np.array(QORD)[None, :, None]
    cc = np.arange(CHA)[None, None, :]
    maskadds = [
        np.where(cc >= (128 * qq + ii - 256 * p), 0.0, -30000.0).astype(bf)
        for p in range(2)
    ]
    pmasks = [np.full((HS, CHA), p, dtype=np.uint8) for p in range(2)]

    in_maps = []
    for core in range(2 * B):
        b, p = core // 2, core % 2
        in_maps.append(
            {
                "xT": xTs[b],
                "wqk": wqk,
                "wv": wv,
                "maskadd": maskadds[p],
                "pmask": pmasks[p],
                "eye": eye,
            }
        )
    return in_maps


def _assemble(results):
    out = np.empty((B, T, HS), dtype=np.float32)
    for core in range(2 * B):
        b, p = core // 2, core % 2
        oc = np.asarray(results[core]["out"], dtype=np.float32)  # [65, 2048]
        for j in range(NSLOT):
            g = 2 * j + p
            blk = oc[:, j * CHA : (j + 1) * CHA]
            out[b, g * CHA : (g + 1) * CHA, :] = (blk[0:HS] / blk[HS : HS + 1]).T
    return out


def run(x, Wk, Wq, Wv, trace=False):
    nc = _get_program()
    in_maps = _host_inputs(x, Wk, Wq, Wv)
    res = run_bass_kernel_spmd(nc, in_maps, list(range(2 * B)), trace=trace)
    return _assemble(res.results), res


def kernel(x, Wk, Wq, Wv):
    out, _ = run(x, Wk, Wq, Wv)
    return out


# revision 24
# speedup vs baseline: 1.0139x; 1.0065x over previous
"""Causal single-head attention on 8 Trainium2 NeuronCores.

Problem: x[4, 4096, 1024], Wq/Wk/Wv[1024, 64] ->
  out = softmax(causal(Q K^T / 8)) V   per batch, fp32.

Sharding: core i handles batch b = i//2 with query-chunk parity p = i%2 at
256-query granularity: core p owns global 256-chunks {2j+p : j=0..7}. Both
cores of a pair load the full x[b] (transposed on host to [C, T]) and
compute full K/V; causal work is balanced by interleaving query chunks.

All matmul operands are bf16 (fp32 accumulate in PSUM). Scores are computed
transposed (S^T[k, q]) with contraction HS=64, so two key tiles run
concurrently in the PE array via partition row-groups (base partition 0 and
64). Each flush step covers a "quad" (4 key tiles = 512 keys x 256 queries)
in one [128, 4, 256] PSUM tile: one exp ACTIVATE per quad. Causality on the
diagonal quad is applied *additively in PSUM before the exp*: 4 extra
matmuls with an identity stationary accumulate -30000 onto masked score
positions, so exp underflows to exactly 0 and no post-exp mask op (or its
latency) exists. V carries a ones column so softmax row-sums accumulate
with P@V; the unnormalized out^T [65, q] is DMA'd straight out of PSUM and
division + transpose happen on host.

Schedule: a short warm-up matmul burst keeps the PE HAM clock-gate at
2.4 GHz through the DMA-limited start; x chunks stream on one strictly
ordered queue so chunk s always lands before chunk s+1 bytes; chunk s's
projections are emitted *inside* the slot s-1 flush (right before the
diagonal PV) so the last exp latency is hidden by projection matmuls.
"""

import numpy as np
import ml_dtypes

import concourse.bacc as bacc
import concourse.mybir as mybir
import concourse.tile as tile
from concourse.bass_utils import run_bass_kernel_spmd

# Problem dims
B, T, C, HS = 4, 4096, 1024, 64
P = 128           # partitions
CH = 512          # projection chunk width
CHA = 256         # attention query-slot width
NSLOT = 8         # query slots per core (256 wide)
CSUB = C // P     # 8 contraction subtiles
NKT = T // P      # 32 key tiles total
NPAIR = NKT // 2  # 16 key-tile pairs
NWARM = 22        # PE warm-up matmuls (HAM clock-gate release)

BF16 = mybir.dt.bfloat16
# key tile (within quad) held by each st/pt slot: slot s <-> tile 4w+QORD[s]
QORD = (1, 3, 0, 2)


def _build_program():
    nc = bacc.Bacc("TRN2")
    f32 = mybir.dt.float32
    EXP = mybir.ActivationFunctionType.Exp

    xT = nc.dram_tensor("xT", [C, T], BF16, kind="ExternalInput").ap()
    wqk = nc.dram_tensor("wqk", [C, 2 * HS], BF16, kind="ExternalInput").ap()
    wv = nc.dram_tensor("wv", [C, HS], BF16, kind="ExternalInput").ap()
    maskadd_d = nc.dram_tensor("maskadd", [P, 4, CHA], BF16, kind="ExternalInput").ap()
    pmask_d = nc.dram_tensor("pmask", [HS, CHA], mybir.dt.uint8, kind="ExternalInput").ap()
    eye_d = nc.dram_tensor("eye", [P, P], BF16, kind="ExternalInput").ap()
    out_d = nc.dram_tensor("out", [HS + 1, NSLOT * CHA], f32, kind="ExternalOutput").ap()

    xT_r = xT.rearrange("(co ci) t -> ci co t", ci=P)      # [128, 8, 4096]
    wqk_r = wqk.rearrange("(co ci) m -> ci co m", ci=P)    # [128, 8, 128]
    wv_r = wv.rearrange("(co ci) m -> ci co m", ci=P)      # [128, 8, 64]

    with tile.TileContext(nc) as tc:
        with (
            tc.tile_pool(name="const", bufs=1) as const_pool,
            tc.tile_pool(name="persist", bufs=1) as persist,
            tc.tile_pool(name="xina", bufs=4) as xpool_a,
            tc.tile_pool(name="xinb", bufs=2) as xpool_b,
            tc.tile_pool(name="pt", bufs=6) as pt_pool,
            tc.tile_pool(name="osb", bufs=2) as osb_pool,
            tc.tile_pool(name="qk_ps", bufs=2, space="PSUM") as qk_pool,
            tc.tile_pool(name="v_ps", bufs=1, space="PSUM") as v_pool,
            tc.tile_pool(name="st_ps", bufs=2, space="PSUM") as st_ps,
            tc.tile_pool(name="ot_ps", bufs=1, space="PSUM") as ot_ps,
        ):
            # ---- constants ----------------------------------------------
            # wqk gates the very first projection: give it (and the other
            # small consts) the otherwise-idle Activation queue, leaving the
            # Sync queue exclusively to the strictly-ordered x stream.
            wqk_sb = const_pool.tile([P, CSUB, 2 * HS], BF16)
            wv_sb = const_pool.tile([P, CSUB, HS], BF16)
            maskadd_sb = const_pool.tile([P, 4, CHA], BF16)
            pmask_sb = const_pool.tile([HS, CHA], mybir.dt.uint8)
            eye_sb = const_pool.tile([P, P], BF16)
            nc.scalar.dma_start(wqk_sb[:], wqk_r)
            nc.scalar.dma_start(wv_sb[:], wv_r)
            nc.scalar.dma_start(pmask_sb[:], pmask_d)
            # needed only from the first flush (~iter 1): keep them off the
            # critical const queue
            nc.gpsimd.dma_start(eye_sb[:], eye_d)
            nc.gpsimd.dma_start(maskadd_sb[:], maskadd_d)

            # warm-up operand: all-ones bf16, no DMA dependency
            wtile = const_pool.tile([P, CHA], BF16)
            nc.vector.memset(wtile[:].bitcast(mybir.dt.uint16), 0x3F80)

            # ---- persistent SBUF ---------------------------------------
            # K^T pairs: [0:64, u, :] = tile 2u, [64:128, u, :] = tile 2u+1
            kt_all = persist.tile([P, NPAIR, P], BF16)
            # Q^T, pair-major: slot s at [:, s//2, (s%2)*256:(s%2+1)*256]
            qt_slot = persist.tile([P, NSLOT // 2, 2 * CHA], BF16)
            v_all = persist.tile([P, NKT, HS + 1], BF16)     # V with ones col
            nc.vector.memset(
                v_all[:, :, HS : HS + 1].bitcast(mybir.dt.uint16), 0x3F80
            )

            # ---- x stream: one queue, strict chunk order ----------------
            # chunks 0/1 in fine-grained 1KB-line DMAs (early start for the
            # first projections); chunks 2..7 as 2 MB chunk-pair DMAs whose
            # 2KB-per-partition lines issue ~4x cheaper per byte.
            xtile = {}
            c0 = xpool_a.tile([P, CSUB, CH], BF16, tag="xa")
            for q in range(4):
                nc.sync.dma_start(c0[:, 2 * q : 2 * q + 2, :],
                                  xT_r[:, 2 * q : 2 * q + 2, 0:CH])
            c1 = xpool_a.tile([P, CSUB, CH], BF16, tag="xa")
            for q in range(2):
                nc.sync.dma_start(c1[:, 4 * q : 4 * q + 4, :],
                                  xT_r[:, 4 * q : 4 * q + 4, CH : 2 * CH])
            c2 = xpool_a.tile([P, CSUB, CH], BF16, tag="xa")
            for q in range(2):
                nc.sync.dma_start(c2[:, 4 * q : 4 * q + 4, :],
                                  xT_r[:, 4 * q : 4 * q + 4, 2 * CH : 3 * CH])
            xtile[0], xtile[1], xtile[2] = (c0, 0), (c1, 0), (c2, 0)
            for cp, lo in ((0, 3 * CH), (1, 5 * CH)):
                xc = xpool_b.tile([P, CSUB, 2 * CH], BF16, tag="xb")
                nc.sync.dma_start(xc[:], xT_r[:, :, lo : lo + 2 * CH])
                xtile[3 + 2 * cp] = (xc, 0)
                xtile[4 + 2 * cp] = (xc, CH)
            c7 = xpool_a.tile([P, CSUB, CH], BF16, tag="xa")
            nc.sync.dma_start(c7[:], xT_r[:, :, 7 * CH : 8 * CH])
            xtile[7] = (c7, 0)

            # ---- PE warm-up --------------------------------------------
            # ~3.2us of dummy matmuls while x streams in: holds the PE HAM
            # activity window busy so real matmuls run at 2.4 GHz from the
            # first chunk (otherwise the clock gate stays at 1.2 GHz for the
            # whole DMA-limited start).
            wm = st_ps.tile([P, 4, CHA], f32, tag="st")
            for k in range(NWARM):
                nc.tensor.matmul(
                    wm[:, k % 4, :], lhsT=wtile[:, 0:P], rhs=wtile[:],
                    start=True, stop=True,
                )

            # ---- per-chunk emission helpers -----------------------------
            def emit_qkproj(s):
                xc, xlo = xtile[s]
                qk_ps = qk_pool.tile([P, CH], f32, tag="qk")
                for cs in range(CSUB):
                    nc.tensor.matmul(
                        qk_ps[:],
                        lhsT=wqk_sb[:, cs, :],
                        rhs=xc[:, cs, xlo : xlo + CH],
                        start=(cs == 0),
                        stop=(cs == CSUB - 1),
                    )
                    if s == 0:
                        # chunk 0 is DMA-starved: keep the PE HAM activity
                        # window busy between per-cs slices so the clock
                        # gate never re-throttles during the start
                        for _ in range(2):
                            nc.tensor.matmul(
                                wm[:, cs % 4, :], lhsT=wtile[:, 0:P],
                                rhs=wtile[:], start=True, stop=True,
                            )
                return qk_ps

            def emit_vproj(s):
                xc, xlo = xtile[s]
                v_ps = v_pool.tile([P, 4, HS], f32, tag="v")
                for tt in range(4):
                    for cs in range(CSUB):
                        nc.tensor.matmul(
                            v_ps[:, tt, :],
                            lhsT=xc[:, cs, xlo + tt * P : xlo + (tt + 1) * P],
                            rhs=wv_sb[:, cs, :],
                            start=(cs == 0),
                            stop=(cs == CSUB - 1),
                        )
                return v_ps

            def emit_copies(s, qk_ps, v_ps):
                # Q select for this core's parity (runtime-chosen via pmask)
                qlo = (s % 2) * CHA
                for hb in (0, HS):
                    nc.vector.tensor_copy(
                        qt_slot[hb : hb + HS, s // 2, qlo : qlo + CHA],
                        qk_ps[0:HS, 0:CHA],
                    )
                    nc.vector.copy_predicated(
                        qt_slot[hb : hb + HS, s // 2, qlo : qlo + CHA],
                        pmask_sb[:],
                        qk_ps[0:HS, CHA:CH],
                    )
                # K^T pair split: chunk s holds key tiles 4s..4s+3
                ksrc = qk_ps[HS:P, :].rearrange(
                    "p (i par c) -> p i par c", i=2, par=2, c=P
                )
                nc.vector.tensor_copy(
                    kt_all[0:HS, 2 * s : 2 * s + 2, :], ksrc[:, :, 0, :]
                )
                nc.vector.tensor_copy(
                    kt_all[HS:P, 2 * s : 2 * s + 2, :], ksrc[:, :, 1, :]
                )
                nc.vector.tensor_copy(
                    v_all[:, 4 * s : 4 * s + 4, 0:HS], v_ps[:]
                )

            def emit_squad(j, w):
                # scores S^T for quad w of slot j; diagonal quad (w == j)
                # gets the additive causal mask accumulated in PSUM so exp
                # underflows to 0 on masked positions.
                qlo = (j % 2) * CHA
                st = st_ps.tile([P, 4, CHA], f32, tag="st")
                diag = w == j
                for issue, (slot, o) in enumerate(
                    ((0, 1), (2, 0), (1, 3), (3, 2))
                ):
                    u, hi = divmod(o, 2)  # pair 2w+u, row half hi
                    hb = HS if hi else 0
                    nc.tensor.matmul(
                        st[:, slot, :],
                        lhsT=kt_all[hb : hb + HS, 2 * w + u, :],
                        rhs=qt_slot[hb : hb + HS, j // 2, qlo : qlo + CHA],
                        start=(issue < 2),
                        stop=(issue >= 2) and not diag,
                        skip_group_check=True,
                    )
                if diag:
                    # additive causal mask, 2 slots per matmul (PSUM-bank
                    # sized N=512; identity stationary loaded once)
                    for h in range(2):
                        nc.tensor.matmul(
                            st[:, 2 * h : 2 * h + 2, :],
                            lhsT=eye_sb[:],
                            rhs=maskadd_sb[:, 2 * h : 2 * h + 2, :],
                            start=False,
                            stop=True,
                            skip_group_check=True,
                        )
                pt = pt_pool.tile([P, 4, CHA], BF16, tag="pt")
                nc.scalar.activation(pt[:], st[:], EXP, scale=float(HS) ** -0.5)
                return pt

            def emit_pv(j, w, pt, ot):
                for slot, o in ((0, 1), (2, 0), (1, 3), (3, 2)):
                    nc.tensor.matmul(
                        ot[0 : HS + 1, :],
                        lhsT=v_all[:, 4 * w + o, :],
                        rhs=pt[:, slot, :],
                        start=(w == 0 and slot == 0),
                        stop=(w == j and slot == 3),
                    )

            # ---- main loop ---------------------------------------------
            # iter s: flush slot j = s-1 while chunk s's projections are
            # emitted right before the last two PVs (hides the final exps'
            # latency behind projection matmuls). The first AHEAD[j] score
            # quads of slot j run *ahead* in iter j itself, right after
            # chunk j's copies, so the scalar engine's exp stream stays fed
            # across iteration boundaries and the final (projection-less)
            # iteration shrinks.
            AHEAD = [1, 2, 2, 2, 2, 2, 2, 2]
            pts = {}
            for s in range(NSLOT + 1):
                j = s - 1
                if j >= 0:
                    ot = ot_ps.tile([P, CHA], f32, tag="ot")
                    for w in range(AHEAD[j], j + 1):
                        pts[(j, w)] = emit_squad(j, w)
                        if w - 2 >= 0:
                            emit_pv(j, w - 2, pts[(j, w - 2)], ot)
                if s < NSLOT:
                    qk_ps = emit_qkproj(s)
                    v_ps = emit_vproj(s)
                    # copies before the trailing PVs: the vector queue must
                    # reach chunk s's qt select before slot s's first S
                    emit_copies(s, qk_ps, v_ps)
                if j >= 0:
                    if j - 1 >= 0:
                        emit_pv(j, j - 1, pts[(j, j - 1)], ot)
                    emit_pv(j, j, pts[(j, j)], ot)
                    # unnormalized out^T + sums row; host finishes
                    o_sb = osb_pool.tile([HS + 1, CHA], f32, tag="osb")
                    nc.vector.tensor_copy(o_sb[:], ot[0 : HS + 1, :])
                    nc.sync.dma_start(
                        out_d[:, j * CHA : (j + 1) * CHA], o_sb[:]
                    )
                if s < NSLOT:
                    for w in range(AHEAD[s]):
                        pts[(s, w)] = emit_squad(s, w)

    nc.compile()
    return nc


_CACHE = {}


def _get_program():
    if "nc" not in _CACHE:
        _CACHE["nc"] = _build_program()
    return _CACHE["nc"]


def _host_inputs(x, Wk, Wq, Wv):
    bf = ml_dtypes.bfloat16
    x = np.asarray(x, dtype=np.float32)
    wqk = np.ascontiguousarray(
        np.concatenate([np.asarray(Wq), np.asarray(Wk)], axis=1), dtype=np.float32
    ).astype(bf)
    wv = np.ascontiguousarray(np.asarray(Wv), dtype=np.float32).astype(bf)
    eye = np.eye(P, dtype=np.float32).astype(bf)

    xTs = [np.ascontiguousarray(x[b].T).astype(bf) for b in range(B)]

    # kept[i, q, c] = 1 iff c >= 128*QORD[q] + i - 256 p   (diagonal quad);
    # additive mask: -30000 where not kept, 0 where kept (pre-exp, in PSUM)
    ii = np.arange(P)[:, None, None]
    qq = np.array(QORD)[None, :, None]
    cc = np.arange(CHA)[None, None, :]
    maskadds = [
        np.where(cc >= (128 * qq + ii - 256 * p), 0.0, -30000.0).astype(bf)
        for p in range(2)
    ]
    pmasks = [np.full((HS, CHA), p, dtype=np.uint8) for p in range(2)]

    in_maps = []
    for core in range(2 * B):
        b, p = core // 2, core % 2
        in_maps.append(
            {
                "xT": xTs[b],
                "wqk": wqk,
                "wv": wv,
                "maskadd": maskadds[p],
                "pmask": pmasks[p],
                "eye": eye,
            }
        )
    return in_maps


def _assemble(results):
    out = np.empty((B, T, HS), dtype=np.float32)
    for core in range(2 * B):
        b, p = core // 2, core % 2
        oc = np.asarray(results[core]["out"], dtype=np.float32)  # [65, 2048]
        for j in range(NSLOT):
            g = 2 * j + p
            blk = oc[:, j * CHA : (j + 1) * CHA]
            out[b, g * CHA : (g + 1) * CHA, :] = (blk[0:HS] / blk[HS : HS + 1]).T
    return out


def run(x, Wk, Wq, Wv, trace=False):
    nc = _get_program()
    in_maps = _host_inputs(x, Wk, Wq, Wv)
    res = run_bass_kernel_spmd(nc, in_maps, list(range(2 * B)), trace=trace)
    return _assemble(res.results), res


def kernel(x, Wk, Wq, Wv):
    out, _ = run(x, Wk, Wq, Wv)
    return out
